# revision 32
# baseline (speedup 1.0000x reference)
"""Trainium2 Bass kernel for DepthEmissionRaymarcher.

Sharding: fully data-parallel over rays. The (B,H,W) leading dims are
flattened to 81920 rays and split evenly across 8 NeuronCores (10240
rays/core). Each core processes its rays in tiles of NB*128 rays (NB
ray-blocks of 128 on the partition axis). No cross-core communication.

Per-ray math (matches the jax reference up to fp32 sum order):
  d = densities, with the last sample forced opaque and samples beyond
      the floor-plane crossing depth forced opaque
  c = min(cumsum(d), 1)      (clamped-state scan == clamped cumsum, d>=0)
  probs = diff(c, prepend=0)
  depth = sum(probs * lengths)
  features = sum(probs[:, None] * rays_features, axis=P)

Early-termination: once the clamped cumulative density reaches 1, every
later sample has probs == 0 and contributes nothing. A host-side float64
check (with margin) finds the rays that saturate within the first K=8
samples — all but ~P(sum of 8 U[0,1) < 1) = 1/8! ~ 2.5e-5 of them for any
non-degenerate density field. The device kernel reads only the first 8
samples of every per-sample tensor (8x less DMA + vector work, clamping
bit-identical to full P for passing rays); the handful of failing rays are
recomputed exactly on the host afterwards. If an adversarial input makes
too many rays fail, everything falls back to a full-P device kernel (same
builder, forced-last-sample variant). Depth is folded into the feature
reduction as a 33rd feature column (DEPTH_FOLD), so the tree-add emits
sum(probs*lengths) for free.

On-chip layout is ray-major (128 rays on partitions). The feature
reduction is a unit-stride broadcast multiply followed by a halving
tree-add over the sample axis, all on the DVE (GpSimd shares the DVE's
SBUF port pair and blocks it outright, and the PE path loses to per-chunk
LDWEIGHTS cost, so neither can usefully offload the reduction).
densities+lengths are packed host-side into one [N, 2K] tensor, preloaded
for all tiles on the ACT HWDGE ring at startup so the per-tile compute
never waits behind a 1 MB feature DMA on the SP ring.
"""

import sys

for _p in (
    "/root/.axon_site",
    "/root/.axon_site/_ro/trn_rl_repo",
    "/root/.axon_site/_ro/pypackages",
):
    if _p not in sys.path:
        sys.path.append(_p)

from contextlib import ExitStack

import numpy as np

import concourse.bass as bass
import concourse.tile as tile
from concourse import bacc, bass_utils, mybir
from concourse.bass_interp import get_hw_module

B, H, W, P, F = 2, 160, 256, 64, 32
NCORES = 8
NRAYS = B * H * W            # 81920
RPC = NRAYS // NCORES        # 10240 rays per core
NBLOCKS = RPC // 128         # 80 ray-blocks of 128
K_TRUNC = 4                  # truncated sample count (power of two)
F32 = mybir.dt.float32
A = mybir.AluOpType

# tuning knobs: blocks per tile, engine per tree-add level (largest first).
# NOTE: concurrent GpSimd + DVE tensor ops contend for SBUF ports (the DVE
# stalls to a crawl while a Pool add runs), so the tree stays on DVE.
NB_BY_P = {4: 16, 8: 16, 16: 8, 64: 2}
TREE_ASSIGN = {4: "vv", 8: "vvv", 16: "vvvv", 64: "vvvvvv"}
BUFS_BY_P = {4: 4, 8: 3, 16: 3, 64: 2}
# K=16 fast path: do the sample-axis reduction on the PE instead of a DVE
# tree — PE and ACT have their own SBUF ports, so this runs fully parallel
# to the DVE. PE transposes each [128,128] chunk of the weighted features
# into PSUM, ACT copies them back to SBUF, then per-chunk matmuls with a
# [128,32] f-selection matrix accumulate sum-over-p directly in [ray, f]
# layout in PSUM, which is DMA'd straight to DRAM.
PE_REDUCE = {4: False, 8: False, 16: False, 64: False}
# fold depth into the feature reduction: lengths ride along as a 33rd
# feature column, so the tree-add produces sum(probs*lengths) for free and
# the per-block depth accumulation ops disappear.
DEPTH_FOLD = {4: True, 8: True, 16: False, 64: False}


def build_body(ctx, tc, dl, feat, ozT, dzT, ident, sel, depthT_o, featout_o,
               p_eff, forced_last):
    nc = tc.nc
    K = p_eff
    NB = NB_BY_P[p_eff]
    FD = F + 1 if DEPTH_FOLD[p_eff] else F
    n_tiles = NBLOCKS // NB

    const = ctx.enter_context(tc.tile_pool(name="const", bufs=1))
    work = ctx.enter_context(tc.tile_pool(name="work", bufs=BUFS_BY_P[p_eff]))

    ones = const.tile([128, K], F32)
    nc.gpsimd.memset(ones[:], 1.0)

    # --- prologue: effective floor depth per ray -------------------------
    # floor = -oz/dz ; fe = floor if floor > 0 else 1e9 (nan -> nan, which
    # also never clips). Matches the reference's nan_to_num + (<=0 -> 1e4)
    # handling because lengths <= 100. fe is then broadcast K-wide so the
    # per-tile below-floor compare is a plain tensor_tensor op.
    ozt = const.tile([128, NBLOCKS], F32)
    nc.scalar.dma_start(ozt[:], ozT[:])
    dzt = const.tile([128, NBLOCKS], F32)
    nc.scalar.dma_start(dzt[:], dzT[:])
    rz = const.tile([128, NBLOCKS], F32)
    nc.vector.reciprocal(rz[:], dzt[:])
    fl = const.tile([128, NBLOCKS], F32)
    nc.vector.scalar_tensor_tensor(fl[:], ozt[:], -1.0, rz[:], op0=A.mult, op1=A.mult)
    mpos = const.tile([128, NBLOCKS], F32)
    nc.vector.tensor_scalar(mpos[:], fl[:], 0.0, None, op0=A.is_gt)
    fm = const.tile([128, NBLOCKS], F32)
    nc.vector.tensor_tensor(fm[:], fl[:], mpos[:], op=A.mult)
    gneg = const.tile([128, NBLOCKS], F32)
    nc.vector.tensor_scalar(gneg[:], mpos[:], -1.0, 1.0, op0=A.mult, op1=A.add)
    fe = const.tile([128, NBLOCKS], F32)
    nc.vector.scalar_tensor_tensor(fe[:], gneg[:], 1.0e9, fm[:], op0=A.mult, op1=A.add)
    fex = const.tile([128, NBLOCKS * K], F32)
    nc.vector.tensor_copy(
        fex[:].rearrange("r (b k) -> r b k", k=K),
        fe[:].broadcast_to([128, NBLOCKS, K]),
    )

    depthT = const.tile([128, NBLOCKS], F32)

    levels = []
    s = K * FD // 2
    while s >= FD:
        levels.append(s)
        s //= 2
    assign = TREE_ASSIGN[K]
    assert len(assign) == len(levels)
    n_pool = len(assign) - len(assign.lstrip("g"))
    assert assign == "g" * n_pool + "v" * (len(levels) - n_pool)

    state = {}
    pe_reduce = PE_REDUCE[p_eff]
    if pe_reduce:
        ident_t = const.tile([128, 128], F32)
        nc.scalar.dma_start(ident_t[:], ident[:])
        sel_t = const.tile([128, F], F32)
        nc.scalar.dma_start(sel_t[:], sel[:])
        psT = ctx.enter_context(tc.tile_pool(name="psT", bufs=2, space="PSUM"))
        ps2 = ctx.enter_context(tc.tile_pool(name="ps2", bufs=2, space="PSUM"))

    # preload every tile's packed dens+lens rows up front (2.6 MB total)
    # on the ACT ring, so the DVE never waits on a small load queued
    # behind a 1 MB feature DMA in the SP HWDGE FIFO.
    dlpool = ctx.enter_context(tc.tile_pool(name="dl", bufs=1))
    dl_all = dlpool.tile([128, n_tiles * NB * 2 * K], F32)
    # one strided DMA reproducing the per-tile (partition <-> NB consecutive
    # rays) layout for the whole core: 512B runs, single 650ns issue.
    nc.scalar.dma_start(
        dl_all[:].rearrange("r (t j c) -> r t j c", t=n_tiles, j=NB),
        dl.rearrange("(t i j) c -> i t j c", t=n_tiles, i=128),
    )
    dl_tiles = [
        dl_all[:, t * NB * 2 * K : (t + 1) * NB * 2 * K] for t in range(n_tiles)
    ]

    batch_probs = K == 4 and not forced_last
    if batch_probs:
        # the whole probs pipeline for all NBLOCKS ray-blocks in 8 batched
        # ops (runs once, hidden under the first feature-DMA fill):
        # dm = max(d, lens > fe); c = min(cumsum4(dm), 1); pr = diff(c)
        dla = dl_all[:].rearrange("r (b c) -> r b c", c=2 * K)
        bw_all = const.tile([128, NBLOCKS * K], F32)
        bw3 = bw_all[:].rearrange("r (b k) -> r b k", k=K)
        nc.vector.tensor_tensor(
            bw3, dla[:, :, K : 2 * K],
            fex[:].rearrange("r (b k) -> r b k", k=K), op=A.is_gt,
        )
        dm_all = const.tile([128, NBLOCKS * K], F32)
        dm3 = dm_all[:].rearrange("r (b k) -> r b k", k=K)
        nc.vector.tensor_tensor(dm3, bw3, dla[:, :, 0:K], op=A.max)
        td_all = const.tile([128, NBLOCKS * K], F32)
        t3 = td_all[:].rearrange("r (b k) -> r b k", k=K)
        nc.vector.tensor_tensor(
            t3[:, :, 1:4], dm3[:, :, 1:4], dm3[:, :, 0:3], op=A.add
        )
        nc.vector.tensor_copy(t3[:, :, 0:1], dm3[:, :, 0:1])
        c_all = const.tile([128, NBLOCKS * (K + 1)], F32)
        c5 = c_all[:].rearrange("r (b k) -> r b k", k=K + 1)
        nc.vector.memset(c_all[:, 0 :: K + 1], 0.0)
        nc.vector.tensor_tensor(
            c5[:, :, 3:5], t3[:, :, 2:4], t3[:, :, 0:2], op=A.add
        )
        nc.vector.tensor_copy(c5[:, :, 1:3], t3[:, :, 0:2])
        nc.vector.tensor_scalar(
            c5[:, :, 1:5], c5[:, :, 1:5], 1.0, None, op0=A.min
        )
        pr_all = const.tile([128, NBLOCKS * K], F32)
        nc.vector.tensor_tensor(
            pr_all[:].rearrange("r (b k) -> r b k", k=K),
            c5[:, :, 1 : K + 1], c5[:, :, 0:K], op=A.subtract,
        )

    def front(t):
        """DMA loads + probs pipeline + feature multiply (DVE, SP)."""
        rows = slice(t * NB * 128, (t + 1) * NB * 128)
        b0 = t * NB

        dl_t = dl_tiles[t]
        f_t = work.tile([128, NB * K * FD], F32)
        nc.sync.dma_start(f_t[:], feat[rows, :])

        if batch_probs:
            pr = pr_all[:, b0 * K : (b0 + NB) * K]
            tmp = work.tile([128, NB * K * FD], F32)
            nc.vector.scalar_tensor_tensor(
                tmp[:].rearrange("r (j p f) -> r j p f", j=NB, p=K),
                f_t[:].rearrange("r (j p f) -> r j p f", j=NB, p=K),
                1.0,
                pr.rearrange("r (j p) -> r j p", j=NB).broadcast_to(
                    [128, NB, K, FD]
                ),
                op0=A.mult,
                op1=A.mult,
            )
            t1 = work.tile([128, NB * K * FD // 2], F32)
            state[t] = (tmp, t1)
            return

        # densities with below-floor samples forced opaque:
        # dm = max(d, lengths > fe), with fe pre-broadcast K-wide so one
        # pair of ops covers all NB ray-blocks
        dl3 = dl_t[:].rearrange("r (j c) -> r j c", j=NB)
        bw = work.tile([128, NB * K], F32)
        nc.vector.tensor_tensor(
            bw[:].rearrange("r (j k) -> r j k", j=NB),
            dl3[:, :, K : 2 * K],
            fex[:].rearrange("r (b k) -> r b k", k=K)[:, b0 : b0 + NB, :],
            op=A.is_gt,
        )
        dm = work.tile([128, NB * K], F32)
        nc.vector.tensor_tensor(
            dm[:].rearrange("r (j k) -> r j k", j=NB),
            bw[:].rearrange("r (j k) -> r j k", j=NB),
            dl3[:, :, 0:K],
            op=A.max,
        )

        # clamped cumulative density per block, with a leading 0 column; in
        # the forced-last variant the final column is exactly 1.
        c = work.tile([128, NB * (K + 1)], F32)
        nc.vector.memset(c[:, 0 :: K + 1], 0.0)
        if forced_last:
            nc.vector.memset(c[:, K :: K + 1], 1.0)
        dm3 = dm[:].rearrange("r (j k) -> r j k", j=NB)
        c3 = c[:].rearrange("r (j k) -> r j k", j=NB)
        if K == 4 and not forced_last:
            # 4-sample cumsum as two batched doubling steps over all NB
            # blocks at once (the 16 tiny per-block scans are pure op
            # overhead), then one clamp: c = min(cumsum(d), 1).
            tdb = work.tile([128, NB * K], F32)
            t3 = tdb[:].rearrange("r (j k) -> r j k", j=NB)
            nc.vector.tensor_tensor(
                t3[:, :, 1:4], dm3[:, :, 1:4], dm3[:, :, 0:3], op=A.add
            )
            nc.vector.tensor_copy(t3[:, :, 0:1], dm3[:, :, 0:1])
            nc.vector.tensor_tensor(
                c3[:, :, 3:5], t3[:, :, 2:4], t3[:, :, 0:2], op=A.add
            )
            nc.vector.tensor_copy(c3[:, :, 1:3], t3[:, :, 0:2])
            nc.vector.tensor_scalar(
                c3[:, :, 1:5], c3[:, :, 1:5], 1.0, None, op0=A.min
            )
        else:
            for j in range(NB):
                o = j * (K + 1)
                kk = K - 1 if forced_last else K
                nc.vector.tensor_tensor_scan(
                    c[:, o + 1 : o + 1 + kk],
                    dm[:, j * K : j * K + kk],
                    ones[:, 0:kk],
                    0.0,
                    op0=A.add,
                    op1=A.min,
                )

        # per-sample hit probability: probs = c[1:] - c[:-1] (all blocks)
        pr = work.tile([128, NB * K], F32)
        c3 = c[:].rearrange("r (j k) -> r j k", j=NB)
        nc.vector.tensor_tensor(
            pr[:].rearrange("r (j k) -> r j k", j=NB),
            c3[:, :, 1 : K + 1],
            c3[:, :, 0:K],
            op=A.subtract,
        )

        if not DEPTH_FOLD[p_eff]:
            # depth = sum(probs * lengths), one accumulation per ray-block
            sc = work.tile([128, NB * K], F32)
            for j in range(NB):
                nc.vector.scalar_tensor_tensor(
                    sc[:, j * K : (j + 1) * K],
                    pr[:, j * K : (j + 1) * K],
                    1.0,
                    dl_t[:, j * 2 * K + K : (j + 1) * 2 * K],
                    op0=A.mult,
                    op1=A.mult,
                    accum_out=depthT[:, b0 + j : b0 + j + 1],
                )

        # weighted features, natural [j, p, f] layout (all unit-stride);
        # with DEPTH_FOLD, column F of each sample is its length
        tmp = work.tile([128, NB * K * FD], F32)
        nc.vector.scalar_tensor_tensor(
            tmp[:].rearrange("r (j p f) -> r j p f", j=NB, p=K),
            f_t[:].rearrange("r (j p f) -> r j p f", j=NB, p=K),
            1.0,
            pr[:].rearrange("r (j p) -> r j p", j=NB).broadcast_to([128, NB, K, FD]),
            op0=A.mult,
            op1=A.mult,
        )
        if pe_reduce:
            state[t] = (tmp, None)
        else:
            t1 = work.tile([128, NB * K * FD // 2], F32)
            state[t] = (tmp, t1)

    def tree(t, which):
        """Halving tree-add over p within each block; every level is a
        contiguous slab add per block (p is the outer free dim)."""
        tmp, t1 = state[t]
        lo, hi = (0, n_pool) if which == "g" else (n_pool, len(levels))
        src, sstr = (tmp, K * FD) if lo == 0 else (t1, K * FD // 2)
        for li in range(lo, hi):
            s = levels[li]
            eng = nc.gpsimd if assign[li] == "g" else nc.vector
            eng.tensor_tensor(
                t1[:].rearrange("r (j x) -> r j x", j=NB)[:, :, 0:s],
                src[:].rearrange("r (j x) -> r j x", x=sstr)[:, :, 0:s],
                src[:].rearrange("r (j x) -> r j x", x=sstr)[:, :, s : 2 * s],
                op=A.add,
            )
            src, sstr = t1, K * FD // 2

    def tail(t):
        tree(t, "v")
        _, t1 = state.pop(t)
        rows = slice(t * NB * 128, (t + 1) * NB * 128)
        b0 = t * NB
        bstr = K * FD // 2
        if DEPTH_FOLD[p_eff]:
            nc.vector.tensor_copy(
                depthT[:, b0 : b0 + NB], t1[:, F :: bstr]
            )
        nc.scalar.dma_start(
            featout_o[rows, :],
            t1[:].rearrange("r (j x) -> r j x", j=NB)[:, :, 0:F],
        )

    def tail_pe(t):
        """Sum over p on the PE: transpose each [128,128] chunk of tmp to
        PSUM, copy back to SBUF (ACT), then matmul each transposed chunk
        (stationary) against the f-selection matrix, accumulating the four
        p-quarters of each ray-block directly in [ray, (j,f)] PSUM layout."""
        tmp, _ = state.pop(t)
        rows = slice(t * NB * 128, (t + 1) * NB * 128)
        n_chunks = NB * K * F // 128
        per_j = K * F // 128
        half_sz = (n_chunks // 2) * 128
        tmpT = work.tile([128, NB * K * F], F32)
        for half in range(2):
            pT = psT.tile([128, half_sz], F32)
            for ci in range(n_chunks // 2):
                c = half * (n_chunks // 2) + ci
                nc.tensor.transpose(
                    pT[:, ci * 128 : (ci + 1) * 128],
                    tmp[:, c * 128 : (c + 1) * 128],
                    ident_t[:],
                )
            nc.scalar.copy(tmpT[:, half * half_sz : (half + 1) * half_sz], pT[:])
        p2 = ps2.tile([128, NB * F], F32)
        for c in range(n_chunks):
            j, q = c // per_j, c % per_j
            nc.tensor.matmul(
                p2[:, j * F : (j + 1) * F],
                tmpT[:, c * 128 : (c + 1) * 128],
                sel_t[:],
                start=(q == 0),
                stop=(q == per_j - 1),
            )
        fo = work.tile([128, NB * F], F32)
        nc.scalar.copy(fo[:], p2[:])
        nc.scalar.dma_start(featout_o[rows, :], fo[:])

    # software pipeline: downstream engines run one/two tiles behind the
    # DVE front so no engine waits on same-tile cross-engine work. With no
    # Pool tree levels everything is DVE in-order anyway — no skew needed.
    if pe_reduce:
        for step in range(n_tiles + 1):
            if step < n_tiles:
                front(step)
            if step >= 1:
                tail_pe(step - 1)
    elif n_pool == 0:
        for step in range(n_tiles):
            front(step)
            tail(step)
    else:
        for step in range(n_tiles + 2):
            if step < n_tiles:
                front(step)
            if 1 <= step <= n_tiles:
                tree(step - 1, "g")
            if step >= 2:
                tail(step - 2)

    nc.scalar.dma_start(depthT_o[:], depthT[:])


_CACHED = {}


def build_module(p_eff, forced_last):
    key = (p_eff, forced_last)
    if key in _CACHED:
        return _CACHED[key]
    nc = bacc.Bacc(
        "TRN2",
        target_bir_lowering=False,
        debug=False,
        num_devices=NCORES,
        enable_asserts=False,
    )
    R = RPC
    dl = nc.dram_tensor("dl", [R, 2 * p_eff], F32, kind="ExternalInput").ap()
    fd = F + 1 if DEPTH_FOLD[p_eff] else F
    feat = nc.dram_tensor("feat", [R, p_eff * fd], F32, kind="ExternalInput").ap()
    ozT = nc.dram_tensor("ozT", [128, NBLOCKS], F32, kind="ExternalInput").ap()
    dzT = nc.dram_tensor("dzT", [128, NBLOCKS], F32, kind="ExternalInput").ap()
    depthT_o = nc.dram_tensor("depthT", [128, NBLOCKS], F32, kind="ExternalOutput").ap()
    featout_o = nc.dram_tensor("featout", [R, F], F32, kind="ExternalOutput").ap()
    ident = sel = None
    if PE_REDUCE[p_eff]:
        ident = nc.dram_tensor("ident", [128, 128], F32, kind="ExternalInput").ap()
        sel = nc.dram_tensor("sel", [128, F], F32, kind="ExternalInput").ap()

    with tile.TileContext(nc) as tc:
        with ExitStack() as ctx:
            build_body(ctx, tc, dl, feat, ozT, dzT, ident, sel, depthT_o,
                       featout_o, p_eff, forced_last)

    nc.compile()
    nc.m = get_hw_module(nc.m)
    _CACHED[key] = nc
    return nc


def _floor_depth(oz, dz):
    with np.errstate(divide="ignore", invalid="ignore"):
        fd = -oz.astype(np.float64) / dz.astype(np.float64)
    return np.where(np.isnan(fd) | (fd <= 0.0), 1e30, fd)


def _truncation_mask(dens, lens, oz, dz, k):
    """Per-ray: does the (floor-clipped) density cumsum reach 1 within the
    first k samples, with enough margin that the device fp32 clamp
    saturates too (=> truncating that ray to k samples is exact)?"""
    fd = _floor_depth(oz, dz)
    d = dens[:, :k].astype(np.float64)
    d = np.where(lens[:, :k].astype(np.float64) > fd[:, None], 1.0, d)
    return d.sum(axis=1) >= 1.0 + 1e-3


def _host_exact(idx, dens, lens, oz, dz, feats):
    """Exact full-P reference math for a handful of rays (numpy float64)."""
    d = dens[idx].astype(np.float64)
    d[:, -1] = 1.0
    fd = _floor_depth(oz[idx], dz[idx])
    ln = lens[idx].astype(np.float64)
    d = np.where(ln > fd[:, None], 1.0, d)
    c = np.minimum(np.cumsum(d, axis=1), 1.0)
    probs = np.diff(c, axis=1, prepend=0.0)
    depth = (probs * ln).sum(axis=1)
    feats = (probs[:, :, None] * feats[idx].astype(np.float64)).sum(axis=1)
    return depth.astype(np.float32), feats.astype(np.float32)


def make_in_maps(rays_densities, rays_features, origins, directions, lengths,
                 p_eff):
    NB = NB_BY_P[p_eff]
    dens = np.asarray(rays_densities, dtype=np.float32).reshape(NRAYS, P)
    feat = np.asarray(rays_features, dtype=np.float32).reshape(NRAYS, P, F)
    lens = np.asarray(lengths, dtype=np.float32).reshape(NRAYS, P)
    dl = np.concatenate([dens[:, :p_eff], lens[:, :p_eff]], axis=1)
    dl = np.ascontiguousarray(dl, dtype=np.float32)
    if p_eff != P:
        feat = feat[:, :p_eff, :]
    if DEPTH_FOLD[p_eff]:
        feat = np.concatenate([feat, lens[:, :p_eff, None]], axis=2)
    fd = feat.shape[2]
    feat = np.ascontiguousarray(feat, dtype=np.float32).reshape(NRAYS, p_eff * fd)
    oz = np.asarray(origins, dtype=np.float32).reshape(NRAYS, 3)[:, 2]
    dz = np.asarray(directions, dtype=np.float32).reshape(NRAYS, 3)[:, 2]

    # ray r of a core sits at tile t = r // (NB*128), partition i, slot j
    # with r = NB*128*t + NB*i + j (a contiguous DMA fills partitions with
    # NB consecutive DRAM rows each). Column index for per-ray scalars is
    # b = NB*t + j.
    def to_cols(x):
        n_tiles = NBLOCKS // NB
        return np.ascontiguousarray(
            x.reshape(n_tiles, 128, NB).transpose(1, 0, 2).reshape(128, NBLOCKS)
        )

    extra = {}
    if PE_REDUCE[p_eff]:
        extra["ident"] = np.eye(128, dtype=np.float32)
        extra["sel"] = np.tile(np.eye(F, dtype=np.float32), (128 // F, 1))

    in_maps = []
    for c in range(NCORES):
        lo, hi = c * RPC, (c + 1) * RPC
        in_maps.append(
            {
                "dl": dl[lo:hi],
                "feat": feat[lo:hi],
                "ozT": to_cols(oz[lo:hi]),
                "dzT": to_cols(dz[lo:hi]),
                **extra,
            }
        )
    return in_maps


def run(in_maps, p_eff, trace=False, **kwargs):
    nc = build_module(p_eff, p_eff == P)
    return bass_utils.run_bass_kernel_spmd(
        nc, in_maps, core_ids=list(range(NCORES)), trace=trace, **kwargs
    )


def kernel(rays_densities, rays_features, origins, directions, lengths):
    dens2 = np.asarray(rays_densities, dtype=np.float32).reshape(NRAYS, P)
    lens2 = np.asarray(lengths, dtype=np.float32).reshape(NRAYS, P)
    oz = np.asarray(origins, dtype=np.float32).reshape(NRAYS, 3)[:, 2]
    dz = np.asarray(directions, dtype=np.float32).reshape(NRAYS, 3)[:, 2]
    # rays that don't saturate within K_TRUNC samples are patched exactly on
    # the host afterwards; fall back to the full-P device kernel only if
    # there are enough of them to matter (degenerate density fields).
    ok = _truncation_mask(dens2, lens2, oz, dz, K_TRUNC)
    nbad = int((~ok).sum())
    p_eff = K_TRUNC if nbad <= 8192 else P

    in_maps = make_in_maps(
        rays_densities, rays_features, origins, directions, lengths, p_eff
    )
    res = run(in_maps, p_eff)
    depth = np.empty((NRAYS,), np.float32)
    features = np.empty((NRAYS, F), np.float32)
    n_tiles = NBLOCKS // NB_BY_P[p_eff]
    for c, r in enumerate(res.results):
        lo, hi = c * RPC, (c + 1) * RPC
        # depthT[i, NB*t + j] is the depth of ray NB*128*t + NB*i + j
        depth[lo:hi] = (
            r["depthT"]
            .reshape(128, n_tiles, NB_BY_P[p_eff])
            .transpose(1, 0, 2)
            .reshape(-1)
        )
        features[lo:hi] = r["featout"]
    if p_eff != P and nbad:
        idx = np.nonzero(~ok)[0]
        feats2 = np.asarray(rays_features, dtype=np.float32).reshape(NRAYS, P, F)
        depth[idx], features[idx] = _host_exact(idx, dens2, lens2, oz, dz, feats2)
    return depth.reshape(B, H, W), features.reshape(B, H, W, F)


# revision 33
# speedup vs baseline: 1.1107x; 1.1107x over previous
"""Trainium2 Bass kernel for DepthEmissionRaymarcher.

Sharding: fully data-parallel over rays. The (B,H,W) leading dims are
flattened to 81920 rays and split evenly across 8 NeuronCores (10240
rays/core). Each core processes its rays in tiles of NB*128 rays (NB
ray-blocks of 128 on the partition axis). No cross-core communication.

Per-ray math (matches the jax reference up to fp32 sum order):
  d = densities, with the last sample forced opaque and samples beyond
      the floor-plane crossing depth forced opaque
  c = min(cumsum(d), 1)      (clamped-state scan == clamped cumsum, d>=0)
  probs = diff(c, prepend=0)
  depth = sum(probs * lengths)
  features = sum(probs[:, None] * rays_features, axis=P)

Early-termination: once the clamped cumulative density reaches 1, every
later sample has probs == 0 and contributes nothing. A host-side float64
check (with margin) finds the rays that saturate within the first K=8
samples — all but ~P(sum of 8 U[0,1) < 1) = 1/8! ~ 2.5e-5 of them for any
non-degenerate density field. The device kernel reads only the first 8
samples of every per-sample tensor (8x less DMA + vector work, clamping
bit-identical to full P for passing rays); the handful of failing rays are
recomputed exactly on the host afterwards. If an adversarial input makes
too many rays fail, everything falls back to a full-P device kernel (same
builder, forced-last-sample variant). Depth is folded into the feature
reduction as a 33rd feature column (DEPTH_FOLD), so the tree-add emits
sum(probs*lengths) for free.

On-chip layout is ray-major (128 rays on partitions). The feature
reduction is a unit-stride broadcast multiply followed by a halving
tree-add over the sample axis, all on the DVE (GpSimd shares the DVE's
SBUF port pair and blocks it outright, and the PE path loses to per-chunk
LDWEIGHTS cost, so neither can usefully offload the reduction).
densities+lengths are packed host-side into one [N, 2K] tensor, preloaded
for all tiles on the ACT HWDGE ring at startup so the per-tile compute
never waits behind a 1 MB feature DMA on the SP ring.
"""

import sys

for _p in (
    "/root/.axon_site",
    "/root/.axon_site/_ro/trn_rl_repo",
    "/root/.axon_site/_ro/pypackages",
):
    if _p not in sys.path:
        sys.path.append(_p)

from contextlib import ExitStack

import numpy as np

import concourse.bass as bass
import concourse.tile as tile
from concourse import bacc, bass_utils, mybir
from concourse.bass_interp import get_hw_module

B, H, W, P, F = 2, 160, 256, 64, 32
NCORES = 8
NRAYS = B * H * W            # 81920
RPC = NRAYS // NCORES        # 10240 rays per core
NBLOCKS = RPC // 128         # 80 ray-blocks of 128
K_TRUNC = 4                  # truncated sample count (power of two)
F32 = mybir.dt.float32
A = mybir.AluOpType

# tuning knobs: blocks per tile, engine per tree-add level (largest first).
# NOTE: concurrent GpSimd + DVE tensor ops contend for SBUF ports (the DVE
# stalls to a crawl while a Pool add runs), so the tree stays on DVE.
NB_BY_P = {4: 16, 8: 16, 16: 8, 64: 2}
TREE_ASSIGN = {4: "vv", 8: "vvv", 16: "vvvv", 64: "vvvvvv"}
BUFS_BY_P = {4: 4, 8: 3, 16: 3, 64: 2}
# K=16 fast path: do the sample-axis reduction on the PE instead of a DVE
# tree — PE and ACT have their own SBUF ports, so this runs fully parallel
# to the DVE. PE transposes each [128,128] chunk of the weighted features
# into PSUM, ACT copies them back to SBUF, then per-chunk matmuls with a
# [128,32] f-selection matrix accumulate sum-over-p directly in [ray, f]
# layout in PSUM, which is DMA'd straight to DRAM.
PE_REDUCE = {4: False, 8: False, 16: False, 64: False}
# fold depth into the feature reduction: lengths ride along as a 33rd
# feature column, so the tree-add produces sum(probs*lengths) for free and
# the per-block depth accumulation ops disappear.
DEPTH_FOLD = {4: True, 8: True, 16: False, 64: False}


def build_body(ctx, tc, dl, feat, ozT, dzT, ident, sel, depthT_o, featout_o,
               p_eff, forced_last):
    nc = tc.nc
    K = p_eff
    NB = NB_BY_P[p_eff]
    FD = F + 1 if DEPTH_FOLD[p_eff] else F
    n_tiles = NBLOCKS // NB

    const = ctx.enter_context(tc.tile_pool(name="const", bufs=1))
    work = ctx.enter_context(tc.tile_pool(name="work", bufs=BUFS_BY_P[p_eff]))

    ones = const.tile([128, K], F32)
    nc.gpsimd.memset(ones[:], 1.0)

    # --- prologue: effective floor depth per ray -------------------------
    # floor = -oz/dz ; fe = floor if floor > 0 else 1e9 (nan -> nan, which
    # also never clips). Matches the reference's nan_to_num + (<=0 -> 1e4)
    # handling because lengths <= 100. fe is then broadcast K-wide so the
    # per-tile below-floor compare is a plain tensor_tensor op.
    ozt = const.tile([128, NBLOCKS], F32)
    nc.scalar.dma_start(ozt[:], ozT[:])
    dzt = const.tile([128, NBLOCKS], F32)
    nc.scalar.dma_start(dzt[:], dzT[:])
    rz = const.tile([128, NBLOCKS], F32)
    nc.vector.reciprocal(rz[:], dzt[:])
    fl = const.tile([128, NBLOCKS], F32)
    nc.vector.scalar_tensor_tensor(fl[:], ozt[:], -1.0, rz[:], op0=A.mult, op1=A.mult)
    mpos = const.tile([128, NBLOCKS], F32)
    nc.vector.tensor_scalar(mpos[:], fl[:], 0.0, None, op0=A.is_gt)
    fm = const.tile([128, NBLOCKS], F32)
    nc.vector.tensor_tensor(fm[:], fl[:], mpos[:], op=A.mult)
    gneg = const.tile([128, NBLOCKS], F32)
    nc.vector.tensor_scalar(gneg[:], mpos[:], -1.0, 1.0, op0=A.mult, op1=A.add)
    fe = const.tile([128, NBLOCKS], F32)
    nc.vector.scalar_tensor_tensor(fe[:], gneg[:], 1.0e9, fm[:], op0=A.mult, op1=A.add)
    fex = const.tile([128, NBLOCKS * K], F32)
    nc.vector.tensor_copy(
        fex[:].rearrange("r (b k) -> r b k", k=K),
        fe[:].broadcast_to([128, NBLOCKS, K]),
    )

    depthT = const.tile([128, NBLOCKS], F32)

    levels = []
    s = K * FD // 2
    while s >= FD:
        levels.append(s)
        s //= 2
    assign = TREE_ASSIGN[K]
    assert len(assign) == len(levels)
    n_pool = len(assign) - len(assign.lstrip("g"))
    assert assign == "g" * n_pool + "v" * (len(levels) - n_pool)

    state = {}
    pe_reduce = PE_REDUCE[p_eff]
    if pe_reduce:
        ident_t = const.tile([128, 128], F32)
        nc.scalar.dma_start(ident_t[:], ident[:])
        sel_t = const.tile([128, F], F32)
        nc.scalar.dma_start(sel_t[:], sel[:])
        psT = ctx.enter_context(tc.tile_pool(name="psT", bufs=2, space="PSUM"))
        ps2 = ctx.enter_context(tc.tile_pool(name="ps2", bufs=2, space="PSUM"))

    # preload every tile's packed dens+lens rows up front (2.6 MB total)
    # on the ACT ring, so the DVE never waits on a small load queued
    # behind a 1 MB feature DMA in the SP HWDGE FIFO.
    dlpool = ctx.enter_context(tc.tile_pool(name="dl", bufs=n_tiles))
    dl_tiles = []
    for t in range(n_tiles):
        rows = slice(t * NB * 128, (t + 1) * NB * 128)
        dl_t = dlpool.tile([128, NB * 2 * K], F32)  # [j, (dens K | lens K)]
        nc.scalar.dma_start(dl_t[:], dl[rows, :])
        dl_tiles.append(dl_t)

    def front(t):
        """DMA loads + probs pipeline + feature multiply (DVE, SP)."""
        rows = slice(t * NB * 128, (t + 1) * NB * 128)
        b0 = t * NB

        dl_t = dl_tiles[t]
        f_t = work.tile([128, NB * K * FD], F32)
        nc.sync.dma_start(f_t[:], feat[rows, :])

        # densities with below-floor samples forced opaque:
        # dm = max(d, lengths > fe), with fe pre-broadcast K-wide so one
        # pair of ops covers all NB ray-blocks
        dl3 = dl_t[:].rearrange("r (j c) -> r j c", j=NB)
        bw = work.tile([128, NB * K], F32)
        nc.vector.tensor_tensor(
            bw[:].rearrange("r (j k) -> r j k", j=NB),
            dl3[:, :, K : 2 * K],
            fex[:].rearrange("r (b k) -> r b k", k=K)[:, b0 : b0 + NB, :],
            op=A.is_gt,
        )
        dm = work.tile([128, NB * K], F32)
        nc.vector.tensor_tensor(
            dm[:].rearrange("r (j k) -> r j k", j=NB),
            bw[:].rearrange("r (j k) -> r j k", j=NB),
            dl3[:, :, 0:K],
            op=A.max,
        )

        # clamped cumulative density per block, with a leading 0 column; in
        # the forced-last variant the final column is exactly 1.
        c = work.tile([128, NB * (K + 1)], F32)
        nc.vector.memset(c[:, 0 :: K + 1], 0.0)
        if forced_last:
            nc.vector.memset(c[:, K :: K + 1], 1.0)
        dm3 = dm[:].rearrange("r (j k) -> r j k", j=NB)
        c3 = c[:].rearrange("r (j k) -> r j k", j=NB)
        if K == 4 and not forced_last:
            # 4-sample cumsum as two batched doubling steps over all NB
            # blocks at once (the 16 tiny per-block scans are pure op
            # overhead), then one clamp: c = min(cumsum(d), 1).
            tdb = work.tile([128, NB * K], F32)
            t3 = tdb[:].rearrange("r (j k) -> r j k", j=NB)
            nc.vector.tensor_tensor(
                t3[:, :, 1:4], dm3[:, :, 1:4], dm3[:, :, 0:3], op=A.add
            )
            nc.vector.tensor_copy(t3[:, :, 0:1], dm3[:, :, 0:1])
            nc.vector.tensor_tensor(
                c3[:, :, 3:5], t3[:, :, 2:4], t3[:, :, 0:2], op=A.add
            )
            nc.vector.tensor_copy(c3[:, :, 1:3], t3[:, :, 0:2])
            nc.vector.tensor_scalar(
                c3[:, :, 1:5], c3[:, :, 1:5], 1.0, None, op0=A.min
            )
        else:
            for j in range(NB):
                o = j * (K + 1)
                kk = K - 1 if forced_last else K
                nc.vector.tensor_tensor_scan(
                    c[:, o + 1 : o + 1 + kk],
                    dm[:, j * K : j * K + kk],
                    ones[:, 0:kk],
                    0.0,
                    op0=A.add,
                    op1=A.min,
                )

        # per-sample hit probability: probs = c[1:] - c[:-1] (all blocks)
        pr = work.tile([128, NB * K], F32)
        c3 = c[:].rearrange("r (j k) -> r j k", j=NB)
        nc.vector.tensor_tensor(
            pr[:].rearrange("r (j k) -> r j k", j=NB),
            c3[:, :, 1 : K + 1],
            c3[:, :, 0:K],
            op=A.subtract,
        )

        if not DEPTH_FOLD[p_eff]:
            # depth = sum(probs * lengths), one accumulation per ray-block
            sc = work.tile([128, NB * K], F32)
            for j in range(NB):
                nc.vector.scalar_tensor_tensor(
                    sc[:, j * K : (j + 1) * K],
                    pr[:, j * K : (j + 1) * K],
                    1.0,
                    dl_t[:, j * 2 * K + K : (j + 1) * 2 * K],
                    op0=A.mult,
                    op1=A.mult,
                    accum_out=depthT[:, b0 + j : b0 + j + 1],
                )

        # weighted features, natural [j, p, f] layout (all unit-stride);
        # with DEPTH_FOLD, column F of each sample is its length
        tmp = work.tile([128, NB * K * FD], F32)
        nc.vector.scalar_tensor_tensor(
            tmp[:].rearrange("r (j p f) -> r j p f", j=NB, p=K),
            f_t[:].rearrange("r (j p f) -> r j p f", j=NB, p=K),
            1.0,
            pr[:].rearrange("r (j p) -> r j p", j=NB).broadcast_to([128, NB, K, FD]),
            op0=A.mult,
            op1=A.mult,
        )
        if pe_reduce:
            state[t] = (tmp, None)
        else:
            t1 = work.tile([128, NB * K * FD // 2], F32)
            state[t] = (tmp, t1)

    def tree(t, which):
        """Halving tree-add over p within each block; every level is a
        contiguous slab add per block (p is the outer free dim)."""
        tmp, t1 = state[t]
        lo, hi = (0, n_pool) if which == "g" else (n_pool, len(levels))
        src, sstr = (tmp, K * FD) if lo == 0 else (t1, K * FD // 2)
        for li in range(lo, hi):
            s = levels[li]
            eng = nc.gpsimd if assign[li] == "g" else nc.vector
            eng.tensor_tensor(
                t1[:].rearrange("r (j x) -> r j x", j=NB)[:, :, 0:s],
                src[:].rearrange("r (j x) -> r j x", x=sstr)[:, :, 0:s],
                src[:].rearrange("r (j x) -> r j x", x=sstr)[:, :, s : 2 * s],
                op=A.add,
            )
            src, sstr = t1, K * FD // 2

    def tail(t):
        tree(t, "v")
        _, t1 = state.pop(t)
        rows = slice(t * NB * 128, (t + 1) * NB * 128)
        b0 = t * NB
        bstr = K * FD // 2
        if DEPTH_FOLD[p_eff]:
            nc.vector.tensor_copy(
                depthT[:, b0 : b0 + NB], t1[:, F :: bstr]
            )
        nc.scalar.dma_start(
            featout_o[rows, :],
            t1[:].rearrange("r (j x) -> r j x", j=NB)[:, :, 0:F],
        )

    def tail_pe(t):
        """Sum over p on the PE: transpose each [128,128] chunk of tmp to
        PSUM, copy back to SBUF (ACT), then matmul each transposed chunk
        (stationary) against the f-selection matrix, accumulating the four
        p-quarters of each ray-block directly in [ray, (j,f)] PSUM layout."""
        tmp, _ = state.pop(t)
        rows = slice(t * NB * 128, (t + 1) * NB * 128)
        n_chunks = NB * K * F // 128
        per_j = K * F // 128
        half_sz = (n_chunks // 2) * 128
        tmpT = work.tile([128, NB * K * F], F32)
        for half in range(2):
            pT = psT.tile([128, half_sz], F32)
            for ci in range(n_chunks // 2):
                c = half * (n_chunks // 2) + ci
                nc.tensor.transpose(
                    pT[:, ci * 128 : (ci + 1) * 128],
                    tmp[:, c * 128 : (c + 1) * 128],
                    ident_t[:],
                )
            nc.scalar.copy(tmpT[:, half * half_sz : (half + 1) * half_sz], pT[:])
        p2 = ps2.tile([128, NB * F], F32)
        for c in range(n_chunks):
            j, q = c // per_j, c % per_j
            nc.tensor.matmul(
                p2[:, j * F : (j + 1) * F],
                tmpT[:, c * 128 : (c + 1) * 128],
                sel_t[:],
                start=(q == 0),
                stop=(q == per_j - 1),
            )
        fo = work.tile([128, NB * F], F32)
        nc.scalar.copy(fo[:], p2[:])
        nc.scalar.dma_start(featout_o[rows, :], fo[:])

    # software pipeline: downstream engines run one/two tiles behind the
    # DVE front so no engine waits on same-tile cross-engine work.
    if pe_reduce:
        for step in range(n_tiles + 1):
            if step < n_tiles:
                front(step)
            if step >= 1:
                tail_pe(step - 1)
    else:
        for step in range(n_tiles + 2):
            if step < n_tiles:
                front(step)
            if 1 <= step <= n_tiles:
                tree(step - 1, "g")
            if step >= 2:
                tail(step - 2)

    nc.scalar.dma_start(depthT_o[:], depthT[:])


_CACHED = {}


def build_module(p_eff, forced_last):
    key = (p_eff, forced_last)
    if key in _CACHED:
        return _CACHED[key]
    nc = bacc.Bacc(
        "TRN2",
        target_bir_lowering=False,
        debug=False,
        num_devices=NCORES,
        enable_asserts=False,
    )
    R = RPC
    dl = nc.dram_tensor("dl", [R, 2 * p_eff], F32, kind="ExternalInput").ap()
    fd = F + 1 if DEPTH_FOLD[p_eff] else F
    feat = nc.dram_tensor("feat", [R, p_eff * fd], F32, kind="ExternalInput").ap()
    ozT = nc.dram_tensor("ozT", [128, NBLOCKS], F32, kind="ExternalInput").ap()
    dzT = nc.dram_tensor("dzT", [128, NBLOCKS], F32, kind="ExternalInput").ap()
    depthT_o = nc.dram_tensor("depthT", [128, NBLOCKS], F32, kind="ExternalOutput").ap()
    featout_o = nc.dram_tensor("featout", [R, F], F32, kind="ExternalOutput").ap()
    ident = sel = None
    if PE_REDUCE[p_eff]:
        ident = nc.dram_tensor("ident", [128, 128], F32, kind="ExternalInput").ap()
        sel = nc.dram_tensor("sel", [128, F], F32, kind="ExternalInput").ap()

    with tile.TileContext(nc) as tc:
        with ExitStack() as ctx:
            build_body(ctx, tc, dl, feat, ozT, dzT, ident, sel, depthT_o,
                       featout_o, p_eff, forced_last)

    nc.compile()
    nc.m = get_hw_module(nc.m)
    _CACHED[key] = nc
    return nc


def _floor_depth(oz, dz):
    with np.errstate(divide="ignore", invalid="ignore"):
        fd = -oz.astype(np.float64) / dz.astype(np.float64)
    return np.where(np.isnan(fd) | (fd <= 0.0), 1e30, fd)


def _truncation_mask(dens, lens, oz, dz, k):
    """Per-ray: does the (floor-clipped) density cumsum reach 1 within the
    first k samples, with enough margin that the device fp32 clamp
    saturates too (=> truncating that ray to k samples is exact)?"""
    fd = _floor_depth(oz, dz)
    d = dens[:, :k].astype(np.float64)
    d = np.where(lens[:, :k].astype(np.float64) > fd[:, None], 1.0, d)
    return d.sum(axis=1) >= 1.0 + 1e-3


def _host_exact(idx, dens, lens, oz, dz, feats):
    """Exact full-P reference math for a handful of rays (numpy float64)."""
    d = dens[idx].astype(np.float64)
    d[:, -1] = 1.0
    fd = _floor_depth(oz[idx], dz[idx])
    ln = lens[idx].astype(np.float64)
    d = np.where(ln > fd[:, None], 1.0, d)
    c = np.minimum(np.cumsum(d, axis=1), 1.0)
    probs = np.diff(c, axis=1, prepend=0.0)
    depth = (probs * ln).sum(axis=1)
    feats = (probs[:, :, None] * feats[idx].astype(np.float64)).sum(axis=1)
    return depth.astype(np.float32), feats.astype(np.float32)


def make_in_maps(rays_densities, rays_features, origins, directions, lengths,
                 p_eff):
    NB = NB_BY_P[p_eff]
    dens = np.asarray(rays_densities, dtype=np.float32).reshape(NRAYS, P)
    feat = np.asarray(rays_features, dtype=np.float32).reshape(NRAYS, P, F)
    lens = np.asarray(lengths, dtype=np.float32).reshape(NRAYS, P)
    dl = np.concatenate([dens[:, :p_eff], lens[:, :p_eff]], axis=1)
    dl = np.ascontiguousarray(dl, dtype=np.float32)
    if p_eff != P:
        feat = feat[:, :p_eff, :]
    if DEPTH_FOLD[p_eff]:
        feat = np.concatenate([feat, lens[:, :p_eff, None]], axis=2)
    fd = feat.shape[2]
    feat = np.ascontiguousarray(feat, dtype=np.float32).reshape(NRAYS, p_eff * fd)
    oz = np.asarray(origins, dtype=np.float32).reshape(NRAYS, 3)[:, 2]
    dz = np.asarray(directions, dtype=np.float32).reshape(NRAYS, 3)[:, 2]

    # ray r of a core sits at tile t = r // (NB*128), partition i, slot j
    # with r = NB*128*t + NB*i + j (a contiguous DMA fills partitions with
    # NB consecutive DRAM rows each). Column index for per-ray scalars is
    # b = NB*t + j.
    def to_cols(x):
        n_tiles = NBLOCKS // NB
        return np.ascontiguousarray(
            x.reshape(n_tiles, 128, NB).transpose(1, 0, 2).reshape(128, NBLOCKS)
        )

    extra = {}
    if PE_REDUCE[p_eff]:
        extra["ident"] = np.eye(128, dtype=np.float32)
        extra["sel"] = np.tile(np.eye(F, dtype=np.float32), (128 // F, 1))

    in_maps = []
    for c in range(NCORES):
        lo, hi = c * RPC, (c + 1) * RPC
        in_maps.append(
            {
                "dl": dl[lo:hi],
                "feat": feat[lo:hi],
                "ozT": to_cols(oz[lo:hi]),
                "dzT": to_cols(dz[lo:hi]),
                **extra,
            }
        )
    return in_maps


def run(in_maps, p_eff, trace=False, **kwargs):
    nc = build_module(p_eff, p_eff == P)
    return bass_utils.run_bass_kernel_spmd(
        nc, in_maps, core_ids=list(range(NCORES)), trace=trace, **kwargs
    )


def kernel(rays_densities, rays_features, origins, directions, lengths):
    dens2 = np.asarray(rays_densities, dtype=np.float32).reshape(NRAYS, P)
    lens2 = np.asarray(lengths, dtype=np.float32).reshape(NRAYS, P)
    oz = np.asarray(origins, dtype=np.float32).reshape(NRAYS, 3)[:, 2]
    dz = np.asarray(directions, dtype=np.float32).reshape(NRAYS, 3)[:, 2]
    # rays that don't saturate within K_TRUNC samples are patched exactly on
    # the host afterwards; fall back to the full-P device kernel only if
    # there are enough of them to matter (degenerate density fields).
    ok = _truncation_mask(dens2, lens2, oz, dz, K_TRUNC)
    nbad = int((~ok).sum())
    p_eff = K_TRUNC if nbad <= 8192 else P

    in_maps = make_in_maps(
        rays_densities, rays_features, origins, directions, lengths, p_eff
    )
    res = run(in_maps, p_eff)
    depth = np.empty((NRAYS,), np.float32)
    features = np.empty((NRAYS, F), np.float32)
    n_tiles = NBLOCKS // NB_BY_P[p_eff]
    for c, r in enumerate(res.results):
        lo, hi = c * RPC, (c + 1) * RPC
        # depthT[i, NB*t + j] is the depth of ray NB*128*t + NB*i + j
        depth[lo:hi] = (
            r["depthT"]
            .reshape(128, n_tiles, NB_BY_P[p_eff])
            .transpose(1, 0, 2)
            .reshape(-1)
        )
        features[lo:hi] = r["featout"]
    if p_eff != P and nbad:
        idx = np.nonzero(~ok)[0]
        feats2 = np.asarray(rays_features, dtype=np.float32).reshape(NRAYS, P, F)
        depth[idx], features[idx] = _host_exact(idx, dens2, lens2, oz, dz, feats2)
    return depth.reshape(B, H, W), features.reshape(B, H, W, F)
